# revision 1
# baseline (speedup 1.0000x reference)
"""GCN encoder (BN -> proj+relu -> GCNConv -> BN -> relu -> GCNConv -> BN)
on 8 Trainium2 NeuronCores via Bass/Tile.

Strategy:
  * Host folds input-BN into the projection weights, computes degree norms,
    and bin-packs destination nodes into 128-row tiles balanced by in-edge
    count (per src-half quotas so gather indices fit int16).  All host-side
    constants ship as ONE packed bf16 "blob" tensor per core (f32/int16
    sections bit-cast on device) -- per-dispatch overhead through the axon
    tunnel scales with argument count (~60us/arg/iteration).
  * Stage 1 is node-sharded: each core computes u1 = D^-1/2 * relu(x@W'+b')
    @ W1 for its own 49 tiles, then one Shared-output AllGather replicates
    the message table (row layout == gather index space, core-major).
  * Edge aggregation: bulk `dma_gather` of message rows spread across TWO
    SWDGE queues (descriptor generation on Q7 cores 0+1 is the bottleneck,
    ~17ns/row; two queues double throughput) + one-hot (is_equal) selection
    matrices built 7-chunks-at-a-time on DVE via stride-0 broadcast APs,
    contracted on the TensorEngine, accumulating each dst tile in PSUM.
  * BN statistics are 2x[1,2F] AllReduces; conv2's message table (computed
    per-owner-core) is exchanged with a second Shared-output AllGather.
"""

import sys

sys.path.insert(0, "/opt/trn_rl_repo")

import heapq

import ml_dtypes
import numpy as np

from concourse import bacc, bass, mybir, tile
from concourse.bass_utils import run_bass_kernel_spmd

P = 128
NCORES = 8
BN_EPS = 1e-5
F32 = mybir.dt.float32
BF16 = mybir.dt.bfloat16
I16 = mybir.dt.int16
AF = mybir.ActivationFunctionType
ALU = mybir.AluOpType
BF16NP = ml_dtypes.bfloat16

SENTINEL = 1000.0  # dstrel value for padding edges (matches no iota entry)


# --------------------------------------------------------------------------
# host-side graph preprocessing
# --------------------------------------------------------------------------

def _pack_half(elo, ehi, ntiles, cap):
    """Pack len(elo) nodes into `ntiles` tiles: <=128 nodes/tile and
    per-half edge sums <= cap.  Greedy LPT with a min-load heap.
    Returns (tile_of, row_of) or None if infeasible."""
    n = len(elo)
    order = np.argsort(-(elo + ehi), kind="stable")
    slo = np.zeros(ntiles, np.int64)
    shi = np.zeros(ntiles, np.int64)
    cnt = np.zeros(ntiles, np.int32)
    tile_of = np.full(n, -1, np.int32)
    row_of = np.full(n, -1, np.int32)
    heap = [(0, t) for t in range(ntiles)]
    heapq.heapify(heap)
    for i in order:
        lo_i = int(elo[i])
        hi_i = int(ehi[i])
        stash = []
        placed = False
        while heap:
            tot, t = heapq.heappop(heap)
            if slo[t] + lo_i <= cap and shi[t] + hi_i <= cap:
                tile_of[i] = t
                row_of[i] = cnt[t]
                cnt[t] += 1
                slo[t] += lo_i
                shi[t] += hi_i
                if cnt[t] < P:
                    heapq.heappush(heap, (int(slo[t] + shi[t]), t))
                placed = True
                break
            stash.append((tot, t))
            if len(stash) > 256:
                break
        for item in stash:
            heapq.heappush(heap, item)
        if not placed:
            return None
    return tile_of, row_of


def _choose_layout(N):
    TPC = -(-N // (NCORES * P))      # dst tiles per core
    if TPC % 2 == 1 and TPC > 1:
        pass  # fine; halves split by tile count NT//2
    NT = NCORES * TPC
    assert NT % 2 == 0
    # gather group size: divisor of TPC closest to 7
    G = 1
    for d in range(1, TPC + 1):
        if TPC % d == 0 and d <= 8:
            G = d
    return TPC, NT, G


def preprocess(x, edge_index, N, IN, H, OUT):
    src = np.asarray(edge_index[0], np.int64)
    dst = np.asarray(edge_index[1], np.int64)
    TPC, NT, G = _choose_layout(N)
    NPAD = NT * P
    HALF_T = NT // 2
    HALF_ROWS = HALF_T * P
    assert HALF_ROWS <= 32767, "gather indices must fit int16"
    NLO = N // 2                      # natural-id boundary between halves

    # degrees (in-degree + self loop), as in PyG gcn_norm
    deg = np.bincount(dst, minlength=N).astype(np.float64) + 1.0
    dinv = (1.0 / np.sqrt(deg)).astype(np.float32)

    # all edges incl. self loops
    s_all = np.concatenate([src, np.arange(N, dtype=np.int64)])
    d_all = np.concatenate([dst, np.arange(N, dtype=np.int64)])
    E_all = len(s_all)
    half_e = (s_all >= NLO).astype(np.int64)

    # per-dst-node lo/hi in-edge counts
    key_lo = d_all[half_e == 0]
    key_hi = d_all[half_e == 1]
    elo = np.bincount(key_lo, minlength=N).astype(np.int64)
    ehi = np.bincount(key_hi, minlength=N).astype(np.int64)

    # pack each natural half of nodes into its half of tiles
    T0 = max(1, -(-int(max(1, E_all // (NT * 2))) // P))
    T_SUB = None
    for T_try in range(T0, T0 + 4):
        cap = T_try * P
        lo_pack = _pack_half(elo[:NLO], ehi[:NLO], HALF_T, cap)
        if lo_pack is None:
            continue
        hi_pack = _pack_half(elo[NLO:], ehi[NLO:], NT - HALF_T, cap)
        if hi_pack is None:
            continue
        T_SUB = T_try
        break
    assert T_SUB is not None, "node packing failed"
    CAP = T_SUB * P

    tile_of = np.empty(N, np.int32)
    row_of = np.empty(N, np.int32)
    tile_of[:NLO] = lo_pack[0]
    row_of[:NLO] = lo_pack[1]
    tile_of[NLO:] = hi_pack[0] + HALF_T
    row_of[NLO:] = hi_pack[1]
    # node -> column in xT / row-order of the per-core output (tile-major)
    pos = tile_of.astype(np.int64) * P + row_of
    # node -> row in the u1/u2 message tables.  Tables are written from a
    # [128, tiles, F] SBUF staging buffer in ONE big DMA, so the row order
    # is partition-major within each core's tile block:
    #   row = core*TPC*128 + p*TPC + t_local
    TPC_ = TPC
    core_of = tile_of // TPC_
    tloc_of = tile_of % TPC_
    tab = (core_of.astype(np.int64) * (TPC_ * P)
           + row_of.astype(np.int64) * TPC_ + tloc_of)

    # edge streams grouped by (dst tile, half)
    t_e = tile_of[d_all].astype(np.int64)
    grp = t_e * 2 + half_e
    order = np.argsort(grp, kind="stable")
    grp_s = grp[order]
    cnts = np.bincount(grp_s, minlength=NT * 2)
    assert cnts.max() <= CAP, f"quota overflow: {cnts.max()} > {CAP}"
    starts = np.zeros(NT * 2, np.int64)
    starts[1:] = np.cumsum(cnts)[:-1]
    within = np.arange(E_all, dtype=np.int64) - starts[grp_s]

    idx_pad = np.zeros((NT, 2, CAP), np.int16)
    rel_pad = np.full((NT, 2, CAP), SENTINEL, np.float32)
    gidx = (tab[s_all] - half_e * HALF_ROWS).astype(np.int16)
    flat = grp_s * CAP + within
    idx_flat = idx_pad.reshape(-1)
    rel_flat = rel_pad.reshape(-1)
    idx_flat[flat] = gidx[order]
    rel_flat[flat] = row_of[d_all][order].astype(np.float32)

    # per-core gather-call index blocks and dstrel columns
    NGRP = TPC // G
    CALL_IDX = G * CAP
    IDXW = CALL_IDX // 16
    idx_maps = []
    rel_maps = []
    dinv_own_maps = []
    dinv_all = np.zeros((P, NT), np.float32)
    valid = row_of >= 0
    dinv_all[row_of[valid], tile_of[valid]] = dinv[valid]
    for c in range(NCORES):
        blocks = []
        rels = []
        for g in range(NGRP):
            t0 = c * TPC + g * G
            for half in range(2):
                blk = idx_pad[t0:t0 + G, half, :].reshape(-1)   # [G*CAP]
                wrapped = blk.reshape(-1, 16).T                  # [16, IDXW]
                blocks.append(np.tile(wrapped, (8, 1)))          # [128, IDXW]
                rb = rel_pad[t0:t0 + G, half, :].reshape(G, T_SUB, P)
                rels.append(np.transpose(rb, (2, 0, 1)).reshape(P, G * T_SUB))
        idx_maps.append(np.ascontiguousarray(np.concatenate(blocks, axis=1)))
        rel_maps.append(np.ascontiguousarray(
            np.concatenate(rels, axis=1).astype(BF16NP)))
        dinv_own_maps.append(np.ascontiguousarray(dinv_all[:, c * TPC:(c + 1) * TPC]))

    # permuted, padded, transposed x
    xp = np.zeros((NPAD, IN), np.float32)
    xp[pos] = x
    xT = np.ascontiguousarray(xp.T.astype(BF16NP))

    cfg = dict(N=N, IN=IN, H=H, OUT=OUT, TPC=TPC, NT=NT, NPAD=NPAD,
               HALF_ROWS=HALF_ROWS, T_SUB=T_SUB, G=G, NGRP=NGRP,
               CALL_IDX=CALL_IDX, IDXW=IDXW)
    host = dict(xT=xT, idx_maps=idx_maps, rel_maps=rel_maps,
                dinv_all=dinv_all, dinv_own_maps=dinv_own_maps, pos=pos)
    return cfg, host


def pack_offsets(cfg):
    """Column offsets for the two packed constant tensors."""
    IN, H, OUT = cfg["IN"], cfg["H"], cfg["OUT"]
    TPC, NT, T_SUB = cfg["TPC"], cfg["NT"], cfg["T_SUB"]
    kb = {}
    kb["w1p"] = 0
    kb["w1c"] = kb["w1p"] + H
    kb["w2c"] = kb["w1c"] + H
    kb["iota"] = kb["w2c"] + OUT
    kb["rel"] = kb["iota"] + T_SUB * 128
    kb["cols"] = kb["rel"] + TPC * 2 * T_SUB
    kf = {}
    kf["ident"] = 0
    kf["dinva"] = kf["ident"] + P
    kf["dinvo"] = kf["dinva"] + NT
    kf["b1p"] = kf["dinvo"] + TPC
    kf["g1"] = kf["b1p"] + 1
    kf["be1"] = kf["g1"] + 1
    kf["g2"] = kf["be1"] + 1
    kf["be2"] = kf["g2"] + OUT
    kf["cols"] = kf["be2"] + OUT
    return kb, kf


def fold_weights(inputs, IN, H, OUT):
    x = np.asarray(inputs["x"], np.float32)
    m0 = x.mean(axis=0, dtype=np.float64)
    v0 = np.mean((x - m0) ** 2, axis=0, dtype=np.float64)
    a = (np.asarray(inputs["bn_in_gamma"], np.float64)
         / np.sqrt(v0 + BN_EPS))
    c = np.asarray(inputs["bn_in_beta"], np.float64) - m0 * a
    projW = np.asarray(inputs["proj_W"], np.float64)
    W1p = (a[:, None] * projW)
    b1p = c @ projW + np.asarray(inputs["proj_b"], np.float64)
    return dict(
        w1p=np.ascontiguousarray(W1p.astype(BF16NP)),
        b1p=np.ascontiguousarray(b1p.astype(np.float32)[:, None]),
        w1c=np.ascontiguousarray(np.asarray(inputs["conv1_W"], np.float32).astype(BF16NP)),
        b1c=np.asarray(inputs["conv1_b"], np.float32),
        w2c=np.ascontiguousarray(np.asarray(inputs["conv2_W"], np.float32).astype(BF16NP)),
        b2c=np.asarray(inputs["conv2_b"], np.float32),
        g1=np.ascontiguousarray(np.asarray(inputs["bn1_gamma"], np.float32)[:, None]),
        be1=np.ascontiguousarray(np.asarray(inputs["bn1_beta"], np.float32)[:, None]),
        g2=np.ascontiguousarray(np.asarray(inputs["bn2_gamma"], np.float32)[None, :]),
        be2=np.ascontiguousarray(np.asarray(inputs["bn2_beta"], np.float32)[None, :]),
    )


# --------------------------------------------------------------------------
# device program
# --------------------------------------------------------------------------

def build_program(cfg, no_cc=False, max_phase=5, conv2_src="u2",
                  agg_mode="full", n_swq=2, single_packet=False,
                  q_of_call=None, dma_scratch=65536, out_bf16=False,
                  gbufs=2):
    IN, H, OUT = cfg["IN"], cfg["H"], cfg["OUT"]
    TPC, NT, NPAD = cfg["TPC"], cfg["NT"], cfg["NPAD"]
    HALF_ROWS, T_SUB, G = cfg["HALF_ROWS"], cfg["T_SUB"], cfg["G"]
    NGRP, CALL_IDX, IDXW = cfg["NGRP"], cfg["CALL_IDX"], cfg["IDXW"]
    N = cfg["N"]
    invN = 1.0 / float(N)
    RG = [list(range(NCORES))]

    kw = {} if dma_scratch is None else dict(dynamic_dma_scratch_size=dma_scratch)
    nc = bacc.Bacc("TRN2", target_bir_lowering=False, debug=False,
                   num_devices=NCORES, num_swdge_queues=n_swq, **kw)

    def inp(name, shape, dty):
        return nc.dram_tensor(name, shape, dty, kind="ExternalInput").ap()

    if max_phase < 0:
        # minimal program: no inputs, tiny output write (dispatch-cost probe)
        out_d = nc.dram_tensor("out", [TPC * P, OUT], F32,
                               kind="ExternalOutput").ap()
        with tile.TileContext(nc) as tc:
            with tc.tile_pool(name="mini", bufs=1) as mpool:
                z = mpool.tile([P, OUT], F32)
                nc.vector.memset(z[:], 0.0)
                nc.sync.dma_start(
                    out_d.rearrange("(t p) f -> p t f", p=P, t=TPC)[:, 0, :],
                    z[:])
        nc.compile()
        return nc

    KB, KF = pack_offsets(cfg)
    IDXC = 2 * NGRP * IDXW
    NOWN = TPC * P                   # rows owned per core
    CONST_COLS = KB["cols"] + 2 * KF["cols"] + IDXC
    blob_d = inp("blob", [P, NOWN + CONST_COLS], BF16)
    xT_d = blob_d[:, 0:NOWN]         # this core's xT slice
    OUT_DT = BF16 if out_bf16 else F32
    out_d = nc.dram_tensor("out", [TPC * P, OUT], OUT_DT,
                           kind="ExternalOutput").ap()

    with tile.TileContext(nc) as tc:
        cpool = tc.alloc_tile_pool(name="const", bufs=1)
        dpool = tc.alloc_tile_pool(name="dram", bufs=1, space="DRAM")

        cc_space = "Local" if no_cc else "Shared"
        u1s_t = dpool.tile([NOWN, H], BF16)
        u1f_t = dpool.tile([NPAD, H], BF16, addr_space=cc_space)
        u2s_t = dpool.tile([TPC * P, P], BF16)
        u2f_t = dpool.tile([NPAD, P], BF16, addr_space=cc_space)
        bn1i = dpool.tile([1, 2 * H], F32)
        bn1o = dpool.tile([1, 2 * H], F32, addr_space=cc_space)
        bn2i = dpool.tile([1, 2 * OUT], F32)
        bn2o = dpool.tile([1, 2 * OUT], F32, addr_space=cc_space)

        def load(name, ap_d, shape, dty):
            t = cpool.tile(shape, dty, tag=name)
            nc.sync.dma_start(t[:], ap_d)
            return t

        kall_s = load("kall", blob_d[:, NOWN:NOWN + CONST_COLS],
                      [P, CONST_COLS], BF16)
        kb_s = kall_s
        kf_s = kall_s[:, KB["cols"]:KB["cols"] + 2 * KF["cols"]].bitcast(F32)
        idx_s = kall_s[:, KB["cols"] + 2 * KF["cols"]:
                       KB["cols"] + 2 * KF["cols"] + IDXC].bitcast(I16)

        w1p_s = kb_s[:, KB["w1p"]:KB["w1p"] + H]
        w1c_s = kb_s[:, KB["w1c"]:KB["w1c"] + H]
        w2c_s = kb_s[:, KB["w2c"]:KB["w2c"] + OUT]
        iota_b = kb_s[:, KB["iota"]:KB["iota"] + P]
        iota7_b = kb_s[:, KB["iota"]:KB["iota"] + T_SUB * P].rearrange(
            "p (j r) -> p j r", j=T_SUB, r=P)
        rel_s = kb_s[:, KB["rel"]:KB["rel"] + TPC * 2 * T_SUB]
        ident_s = kf_s[:, KF["ident"]:KF["ident"] + P]
        dinv_all_s = kf_s[:, KF["dinva"]:KF["dinva"] + NT]
        dinv_own_s = kf_s[:, KF["dinvo"]:KF["dinvo"] + TPC]
        b1p_s = kf_s[:, KF["b1p"]:KF["b1p"] + 1]
        g1_s = kf_s[:, KF["g1"]:KF["g1"] + 1]
        be1_s = kf_s[:, KF["be1"]:KF["be1"] + 1]
        g2_s = kf_s[0:1, KF["g2"]:KF["g2"] + OUT]
        be2_s = kf_s[0:1, KF["be2"]:KF["be2"] + OUT]

        ones_col = cpool.tile([P, 1], F32, tag="onesc")
        nc.vector.memset(ones_col[:], 1.0)
        eps_s = cpool.tile([P, 1], F32, tag="eps")
        nc.vector.memset(eps_s[:], BN_EPS)
        ones_row = cpool.tile([1, P], F32, tag="onesr")
        nc.vector.memset(ones_row[:], 1.0)

        c1_s = cpool.tile([P, TPC, H], F32, tag="c1")
        c2_s = cpool.tile([P, TPC, OUT], F32, tag="c2")

        # ---------------- stage 1: u1 rows for OWN nodes + AllGather ------
        HALF_T = NT // 2
        if max_phase >= 1:
          with tc.tile_pool(name="s1x", bufs=1) as xpool, \
             tc.tile_pool(name="s1h", bufs=4) as hpool, \
             tc.tile_pool(name="s1g", bufs=1) as stgpool, \
             tc.tile_pool(name="s1p", bufs=2, space="PSUM") as pp1, \
             tc.tile_pool(name="s1pu", bufs=4, space="PSUM") as pp2:
            u1_stage = stgpool.tile([P, TPC, H], BF16)
            xt = xpool.tile([IN, NOWN], BF16)
            nc.sync.dma_start(xt[:], xT_d)
            CH = 512
            nch = -(-NOWN // CH)
            for ci in range(nch):
                c0 = ci * CH
                cw = min(CH, NOWN - c0)
                hp = pp1.tile([H, CH], F32)
                nc.tensor.matmul(hp[:, 0:cw], lhsT=w1p_s[:],
                                 rhs=xt[:, c0:c0 + cw],
                                 start=True, stop=True)
                hs = hpool.tile([H, CH], BF16)
                nc.scalar.activation(hs[:, 0:cw], hp[:, 0:cw], AF.Relu,
                                     bias=b1p_s[:, 0:1], scale=1.0)
                for s in range(cw // P):
                    t = (c0 // P) + s
                    up = pp2.tile([P, H], F32)
                    nc.tensor.matmul(up[:],
                                     lhsT=hs[:, s * P:(s + 1) * P],
                                     rhs=w1c_s[:], start=True, stop=True)
                    nc.vector.tensor_scalar_mul(u1_stage[:, t, :], up[:],
                                                dinv_own_s[:, t:t + 1])
            # own-table write: row = p*TPC + t (contiguous per partition)
            nc.sync.dma_start(
                u1s_t[:, :].rearrange("(p t) f -> p t f", p=P, t=TPC),
                u1_stage[:])
            if no_cc:
                for c in range(NCORES):
                    nc.gpsimd.dma_start(
                        u1f_t[c * NOWN:(c + 1) * NOWN, :], u1s_t[:])
            else:
                nc.gpsimd.collective_compute(
                    "AllGather", ALU.bypass, replica_groups=RG,
                    ins=[u1s_t[:].opt()], outs=[u1f_t[:].opt()])

        # ---------------- shared edge aggregation ------------------------
        def aggregate(u_tables, c_store, FC, stat_tag):
            """c_store[:, t, :FC] = dinv * sum_{edges->tile t} u[src];
            returns SBUF [2, FC] tile with [sum; sumsq] partials."""
            with tc.tile_pool(name=f"gb{stat_tag}", bufs=gbufs) as gpool, \
                 tc.tile_pool(name=f"st{stat_tag}", bufs=16) as stpool, \
                 tc.tile_pool(name=f"sq{stat_tag}", bufs=2) as sqpool, \
                 tc.tile_pool(name=f"ap{stat_tag}", bufs=4, space="PSUM") as apool, \
                 tc.tile_pool(name=f"sp{stat_tag}", bufs=1, space="PSUM") as spool:
                sum_p = spool.tile([1, FC], F32, tag="sum")
                sq_p = spool.tile([1, FC], F32, tag="sq")
                for g in range(NGRP):
                    bufs = []
                    for half in range(2):
                        gb = gpool.tile([P, G * T_SUB, P], BF16, tag=f"g{half}")
                        call = g * 2 + half
                        tbl = u_tables[half]
                        nc.gpsimd.dma_gather(
                            out_ap=gb[:],
                            in_ap=tbl,
                            idxs_ap=idx_s[:, call * IDXW:(call + 1) * IDXW],
                            num_idxs=CALL_IDX,
                            num_idxs_reg=CALL_IDX,
                            elem_size=P,
                            single_packet=single_packet,
                            queue_num=(call % n_swq if q_of_call is None
                                       else q_of_call(call)),
                        )
                        bufs.append(gb)
                    for tl in range(G):
                        t = g * G + tl
                        ps = apool.tile([P, FC], F32)
                        if agg_mode == "gonly":
                            nc.tensor.matmul(
                                ps[:], lhsT=iota_b[:],
                                rhs=bufs[0][:, tl * T_SUB, 0:FC],
                                start=True, stop=False)
                            nc.tensor.matmul(
                                ps[:], lhsT=iota_b[:],
                                rhs=bufs[1][:, tl * T_SUB, 0:FC],
                                start=False, stop=True)
                        else:
                            k = 0
                            for half in range(2):
                                c0 = ((g * 2 + half) * (G * T_SUB)
                                      + tl * T_SUB)
                                if agg_mode == "nostt":
                                    stt = None
                                else:
                                    stt = stpool.tile([P, T_SUB, P], BF16)
                                    nc.vector.tensor_tensor(
                                        stt[:],
                                        rel_s[:, c0:c0 + T_SUB]
                                        .to_broadcast([P, T_SUB, P]),
                                        iota7_b,
                                        ALU.is_equal,
                                    )
                                for j in range(T_SUB):
                                    lhsT = (iota_b[:] if stt is None
                                            else stt[:, j, :])
                                    nc.tensor.matmul(
                                        ps[:],
                                        lhsT=lhsT,
                                        rhs=bufs[half][:, tl * T_SUB + j, 0:FC],
                                        start=(k == 0),
                                        stop=(k == 2 * T_SUB - 1),
                                    )
                                    k += 1
                        ctile = c_store[:, t, :]
                        nc.vector.tensor_scalar_mul(ctile, ps[:],
                                                    dinv_own_s[:, t:t + 1])
                        sq = sqpool.tile([P, FC], F32)
                        nc.vector.tensor_mul(sq[:], ctile, ctile)
                        nc.tensor.matmul(sum_p[:], lhsT=ones_col[:],
                                         rhs=ctile,
                                         start=(t == 0), stop=(t == TPC - 1))
                        nc.tensor.matmul(sq_p[:], lhsT=ones_col[:], rhs=sq[:],
                                         start=(t == 0), stop=(t == TPC - 1))
                st_s = cpool.tile([1, 2 * FC], F32, tag=f"stats{stat_tag}")
                nc.vector.tensor_copy(st_s[:, 0:FC], sum_p[:])
                nc.vector.tensor_copy(st_s[:, FC:], sq_p[:])
            return st_s

        def allreduce_stats(st_s, bni, bno, FC, tag):
            nc.sync.dma_start(bni[:], st_s[:])
            if no_cc:
                nc.gpsimd.dma_start(bno[:], bni[:])
            else:
                nc.gpsimd.collective_compute(
                    "AllReduce", ALU.add, replica_groups=RG,
                    ins=[bni[:].opt()], outs=[bno[:].opt()])
            ar = cpool.tile([1, 2 * FC], F32, tag=f"ar{tag}")
            nc.sync.dma_start(ar[:], bno[:])
            return ar

        # ---------------- conv1 + BN1 + relu + u2 table -------------------
        if max_phase >= 2:
            st1 = aggregate((u1f_t[0:HALF_ROWS, :],
                             u1f_t[HALF_ROWS:NPAD, :]), c1_s, H, "c1")
        if max_phase >= 3:
            ar1 = allreduce_stats(st1, bn1i, bn1o, H, "1")
            with tc.tile_pool(name="bn1p", bufs=2, space="PSUM") as bpp, \
                 tc.tile_pool(name="bn1s", bufs=1) as bsp:
                tp_a = bpp.tile([P, 1], F32, tag="tpa")
                nc.tensor.transpose(tp_a[:], ar1[:, 0:H], ident_s[0:1, 0:1])
                tp_b = bpp.tile([P, 1], F32, tag="tpb")
                nc.tensor.transpose(tp_b[:], ar1[:, H:], ident_s[0:1, 0:1])
                mean1 = bsp.tile([P, 1], F32, tag="m1")
                nc.vector.tensor_scalar_mul(mean1[:], tp_a[:], invN)
                msq1 = bsp.tile([P, 1], F32, tag="q1")
                nc.vector.tensor_scalar_mul(msq1[:], tp_b[:], invN)
                var1 = bsp.tile([P, 1], F32, tag="v1")
                nc.vector.tensor_mul(var1[:], mean1[:], mean1[:])
                nc.vector.tensor_tensor(var1[:], msq1[:], var1[:], ALU.subtract)
                std1 = bsp.tile([P, 1], F32, tag="s1d")
                nc.scalar.activation(std1[:], var1[:], AF.Sqrt,
                                     bias=eps_s[:, 0:1])
                inv1 = bsp.tile([P, 1], F32, tag="i1")
                nc.vector.reciprocal(inv1[:], std1[:])
                s1c = cpool.tile([P, 1], F32, tag="s1c")
                nc.vector.tensor_mul(s1c[:], g1_s[:], inv1[:])
                t1tmp = bsp.tile([P, 1], F32, tag="t1t")
                nc.vector.tensor_mul(t1tmp[:], mean1[:], s1c[:])
                t1c = cpool.tile([P, 1], F32, tag="t1c")
                nc.vector.tensor_tensor(t1c[:], be1_s[:], t1tmp[:],
                                        ALU.subtract)

                with tc.tile_pool(name="trp", bufs=2, space="PSUM") as trp, \
                     tc.tile_pool(name="u2p", bufs=2, space="PSUM") as u2p, \
                     tc.tile_pool(name="h2s", bufs=2) as h2pool, \
                     tc.tile_pool(name="u2g", bufs=1) as u2pool:
                    u2stage = u2pool.tile([P, TPC, P], BF16)
                    nc.vector.memset(u2stage[:], 0.0)
                    for t in range(TPC):
                        tp2 = trp.tile([P, P], F32)
                        nc.tensor.transpose(tp2[:], c1_s[:, t, :], ident_s[:])
                        h2t = h2pool.tile([P, P], BF16)
                        nc.scalar.activation(h2t[:], tp2[:], AF.Relu,
                                             bias=t1c[:, 0:1],
                                             scale=s1c[:, 0:1])
                        up2 = u2p.tile([P, OUT], F32)
                        nc.tensor.matmul(up2[:], lhsT=h2t[:], rhs=w2c_s[:],
                                         start=True, stop=True)
                        nc.vector.tensor_scalar_mul(u2stage[:, t, 0:OUT],
                                                    up2[:],
                                                    dinv_own_s[:, t:t + 1])
                    nc.sync.dma_start(
                        u2s_t[:, :].rearrange("(p t) f -> p t f", p=P, t=TPC),
                        u2stage[:])

            if no_cc:
                # timing proxy: 8 local copies stand in for the all-gather
                for c in range(NCORES):
                    nc.gpsimd.dma_start(
                        u2f_t[c * TPC * P:(c + 1) * TPC * P, :], u2s_t[:])
            else:
                nc.gpsimd.collective_compute(
                    "AllGather", ALU.bypass, replica_groups=RG,
                    ins=[u2s_t[:].opt()], outs=[u2f_t[:].opt()])

        # ---------------- conv2 + BN2 + output ----------------------------
        if max_phase >= 4:
            if conv2_src == "u1":
                st2 = aggregate((u1f_t[0:HALF_ROWS, :],
                                 u1f_t[HALF_ROWS:NPAD, :]), c2_s, OUT, "c2")
            else:
                st2 = aggregate((u2f_t[0:HALF_ROWS, :],
                                 u2f_t[HALF_ROWS:NPAD, :]), c2_s, OUT, "c2")
        if max_phase >= 5:
            ar2 = allreduce_stats(st2, bn2i, bn2o, OUT, "2")
            with tc.tile_pool(name="bn2s", bufs=1) as b2p, \
                 tc.tile_pool(name="bn2p", bufs=1, space="PSUM") as b2pp, \
                 tc.tile_pool(name="outp", bufs=3) as opool:
                mean2 = b2p.tile([1, OUT], F32, tag="m2")
                nc.vector.tensor_scalar_mul(mean2[:], ar2[:, 0:OUT], invN)
                msq2 = b2p.tile([1, OUT], F32, tag="q2")
                nc.vector.tensor_scalar_mul(msq2[:], ar2[:, OUT:], invN)
                var2 = b2p.tile([1, OUT], F32, tag="v2")
                nc.vector.tensor_mul(var2[:], mean2[:], mean2[:])
                nc.vector.tensor_tensor(var2[:], msq2[:], var2[:],
                                        ALU.subtract)
                std2 = b2p.tile([1, OUT], F32, tag="s2d")
                nc.scalar.activation(std2[:], var2[:], AF.Sqrt,
                                     bias=eps_s[0:1, 0:1])
                inv2 = b2p.tile([1, OUT], F32, tag="i2")
                nc.vector.reciprocal(inv2[:], std2[:])
                s2r = b2p.tile([1, OUT], F32, tag="s2r")
                nc.vector.tensor_mul(s2r[:], g2_s[:], inv2[:])
                t2tmp = b2p.tile([1, OUT], F32, tag="t2t")
                nc.vector.tensor_mul(t2tmp[:], mean2[:], s2r[:])
                t2r = b2p.tile([1, OUT], F32, tag="t2r")
                nc.vector.tensor_tensor(t2r[:], be2_s[:], t2tmp[:],
                                        ALU.subtract)
                cat = b2p.tile([1, 2 * OUT], F32, tag="cat")
                nc.vector.tensor_copy(cat[:, 0:OUT], s2r[:])
                nc.vector.tensor_copy(cat[:, OUT:], t2r[:])
                bp = b2pp.tile([P, 2 * OUT], F32)
                nc.tensor.matmul(bp[:], lhsT=ones_row[:], rhs=cat[:],
                                 start=True, stop=True)
                rep = b2p.tile([P, 2 * OUT], F32, tag="rep")
                nc.vector.tensor_copy(rep[:], bp[:])
                ostage = opool.tile([P, TPC, OUT], F32)
                for t in range(TPC):
                    nc.vector.tensor_mul(ostage[:, t, :], c2_s[:, t, :],
                                         rep[:, 0:OUT])
                    nc.vector.tensor_add(ostage[:, t, :], ostage[:, t, :],
                                         rep[:, OUT:])
                out_ap = out_d.rearrange("(p t) f -> p t f", p=P, t=TPC)
                if out_bf16:
                    nc.gpsimd.dma_start(out_ap, ostage[:])  # SWDGE f32->bf16
                else:
                    nc.sync.dma_start(out_ap, ostage[:])

        dpool.release()
        cpool.release()

    nc.compile()
    return nc


# --------------------------------------------------------------------------
# runner
# --------------------------------------------------------------------------

def make_in_maps(cfg, host, folded):
    H, OUT, TPC, NT, T_SUB = (cfg["H"], cfg["OUT"], cfg["TPC"], cfg["NT"],
                              cfg["T_SUB"])
    KB, KF = pack_offsets(cfg)
    kb0 = np.zeros((P, KB["cols"]), BF16NP)
    kb0[:, KB["w1p"]:KB["w1p"] + H] = folded["w1p"]
    kb0[:, KB["w1c"]:KB["w1c"] + H] = folded["w1c"]
    kb0[:, KB["w2c"]:KB["w2c"] + OUT] = folded["w2c"]
    kb0[:, KB["iota"]:KB["iota"] + T_SUB * P] = np.tile(
        np.arange(P, dtype=np.float32), T_SUB)[None, :]
    kf0 = np.zeros((P, KF["cols"]), np.float32)
    kf0[:, KF["ident"]:KF["ident"] + P] = np.eye(P, dtype=np.float32)
    kf0[:, KF["dinva"]:KF["dinva"] + NT] = host["dinv_all"]
    kf0[:, KF["b1p"]] = folded["b1p"][:, 0]
    kf0[:, KF["g1"]] = folded["g1"][:, 0]
    kf0[:, KF["be1"]] = folded["be1"][:, 0]
    kf0[0, KF["g2"]:KF["g2"] + OUT] = folded["g2"][0]
    kf0[0, KF["be2"]:KF["be2"] + OUT] = folded["be2"][0]
    NGRP, IDXW = cfg["NGRP"], cfg["IDXW"]
    IDXC = 2 * NGRP * IDXW
    NOWN = TPC * P
    CONST_COLS = KB["cols"] + 2 * KF["cols"] + IDXC
    in_maps = []
    for c in range(NCORES):
        kb = kb0.copy()
        kb[:, KB["rel"]:KB["rel"] + TPC * 2 * T_SUB] = host["rel_maps"][c]
        kf = kf0.copy()
        kf[:, KF["dinvo"]:KF["dinvo"] + TPC] = host["dinv_own_maps"][c]
        blob = np.empty((P, NOWN + CONST_COLS), BF16NP)
        blob[:, 0:NOWN] = host["xT"][:, c * NOWN:(c + 1) * NOWN]
        o = NOWN
        blob[:, o:o + KB["cols"]] = kb
        o += KB["cols"]
        blob[:, o:o + 2 * KF["cols"]] = np.ascontiguousarray(kf).view(BF16NP)
        o += 2 * KF["cols"]
        blob[:, o:o + IDXC] = host["idx_maps"][c].view(BF16NP)
        in_maps.append(dict(blob=blob))
    return in_maps


def assemble_output(cfg, host, results):
    TPC, OUT, N = cfg["TPC"], cfg["OUT"], cfg["N"]
    # per-core "out" rows are ordered row = p*TPC + t; convert to
    # tile-major (t*128 + p) order, then apply the node permutation
    parts = [results[c]["out"].reshape(P, TPC, OUT).transpose(1, 0, 2)
             .reshape(TPC * P, OUT) for c in range(NCORES)]
    full = np.concatenate(parts, axis=0)
    return np.ascontiguousarray(full[host["pos"][:N]], dtype=np.float32)


_PROGRAM_CACHE = {}


def _get_program(cfg):
    key = tuple(sorted(cfg.items()))
    if key not in _PROGRAM_CACHE:
        _PROGRAM_CACHE[key] = build_program(cfg)
    return _PROGRAM_CACHE[key]


def run(inputs, trace=False):
    x = np.asarray(inputs["x"], np.float32)
    N, IN = x.shape
    H = np.asarray(inputs["conv1_W"]).shape[0]
    OUT = np.asarray(inputs["conv2_W"]).shape[1]
    cfg, host = preprocess(x, inputs["edge_index"], N, IN, H, OUT)
    folded = fold_weights(inputs, IN, H, OUT)
    nc = _get_program(cfg)
    in_maps = make_in_maps(cfg, host, folded)
    res = run_bass_kernel_spmd(nc, in_maps, list(range(NCORES)), trace=trace)
    out = assemble_output(cfg, host, res.results)
    return out, res


def kernel(**inputs) -> np.ndarray:
    out, _ = run(inputs, trace=False)
    return out


# --------------------------------------------------------------------------
# benchmarking (repeated execution of the compiled NEFF via PJRT)
# --------------------------------------------------------------------------

def bench(inputs, iters=16, nc=None, warmup=8):
    """Time back-to-back executions of the compiled program with inputs
    pre-staged on device.  Returns (ns_per_iter, output)."""
    import time

    import jax
    import numpy as jnp_np
    from concourse import bass2jax, mybir as mb

    x = np.asarray(inputs["x"], np.float32)
    N, IN = x.shape
    H = np.asarray(inputs["conv1_W"]).shape[0]
    OUT = np.asarray(inputs["conv2_W"]).shape[1]
    cfg, host = preprocess(x, inputs["edge_index"], N, IN, H, OUT)
    folded = fold_weights(inputs, IN, H, OUT)
    if nc is None:
        nc = _get_program(cfg)
    in_maps = make_in_maps(cfg, host, folded)

    bass2jax.install_neuronx_cc_hook()
    partition_name = (nc.partition_id_tensor.name
                      if nc.partition_id_tensor else None)
    in_names, out_names, out_avals, zero_outs = [], [], [], []
    for alloc in nc.m.functions[0].allocations:
        if not isinstance(alloc, mb.MemoryLocationSet):
            continue
        name = alloc.memorylocations[0].name
        if alloc.kind == "ExternalInput":
            if name != partition_name:
                in_names.append(name)
        elif alloc.kind == "ExternalOutput":
            out_avals.append(jax.core.ShapedArray(
                tuple(alloc.tensor_shape), mb.dt.np(alloc.dtype)))
            out_names.append(name)
            zero_outs.append(np.zeros(alloc.tensor_shape,
                                      mb.dt.np(alloc.dtype)))
    n_params = len(in_names)
    all_in_names = in_names + out_names
    if partition_name is not None:
        all_in_names.append(partition_name)

    def _body(*args):
        operands = list(args)
        if partition_name is not None:
            operands.append(bass2jax.partition_id_tensor())
        outs = bass2jax._bass_exec_p.bind(
            *operands,
            out_avals=tuple(out_avals),
            in_names=tuple(all_in_names),
            out_names=tuple(out_names),
            lowering_input_output_aliases=(),
            sim_require_finite=True,
            sim_require_nnan=True,
            nc=nc,
        )
        return tuple(outs)

    devices = jax.devices()[:NCORES]
    mesh = bass2jax.Mesh(np.asarray(devices), ("core",))
    in_specs = (bass2jax.PartitionSpec("core"),) * (n_params + len(out_names))
    out_specs = (bass2jax.PartitionSpec("core"),) * len(out_names)
    sharded = jax.jit(bass2jax.shard_map(
        _body, mesh=mesh, in_specs=in_specs, out_specs=out_specs,
        check_rep=False))

    concat_in = [np.concatenate([np.asarray(in_maps[c][nm])
                                 for c in range(NCORES)], axis=0)
                 for nm in in_names]
    concat_zeros = [np.zeros((NCORES * z.shape[0], *z.shape[1:]), z.dtype)
                    for z in zero_outs]
    from jax.sharding import NamedSharding
    sh = NamedSharding(mesh, bass2jax.PartitionSpec("core"))
    dev_in = [jax.device_put(a, sh) for a in concat_in]
    dev_zeros = [jax.device_put(a, sh) for a in concat_zeros]

    for _ in range(max(1, warmup)):
        out_arrs = sharded(*dev_in, *dev_zeros)
    jax.block_until_ready(out_arrs)  # warmup + compile + pipeline ramp
    dt_ns = None
    for _ in range(20):              # repeat batches; report best (timeit-style)
        t0 = time.perf_counter()
        for _ in range(iters):
            out_arrs = sharded(*dev_in, *dev_zeros)
        jax.block_until_ready(out_arrs)
        batch_ns = (time.perf_counter() - t0) / iters * 1e9
        if dt_ns is None or batch_ns < dt_ns:
            dt_ns = batch_ns

    results = [
        {name: np.asarray(out_arrs[i]).reshape(NCORES, *out_avals[i].shape)[c]
         for i, name in enumerate(out_names)}
        for c in range(NCORES)
    ]
    out = assemble_output(cfg, host, results)
    return dt_ns, out

